# revision 6
# baseline (speedup 1.0000x reference)
"""Trainium2 Bass kernel for nn_CNF: per-sample CNF log-likelihood.

Computes, for each sample y0 in the batch, the Tsit5 (fixed step, 10 steps,
6 stages) integration of (y, dlogp) backwards through 2 ConcatSquash-MLP
blocks with EXACT Jacobian-trace divergence, then adds the standard-normal
log-density.  Output shape [4096] fp32.

Sharding: pure data parallel over 8 NeuronCores (512 samples each); all
parameters replicated; the whole integration runs on-device in one NEFF.

Math restructuring (validated against the jax reference in mirror.py):
  * t is sample-independent -> all gate vectors g_l = sigmoid(W2_l t + b2_l)
    and d_l = b1_l*g_l + W3_l t are host-precomputed per (block, eval).
  * forward (feature-major [feat, sample]):
      h_l = g_l * (W1_l @ y) + d_l ; y = softplus(h_l) (synthesized from the
      exp/ln ACT table set: softplus = relu(h) + ln(1+exp(-|h|)),
      sigmoid = exp(min(h,0) - ln(1+exp(-|h|)))).
  * Tsit5 stage states are never materialized: stage-s L1 input is the
    K-stacked [y;ky0..] bank multiplied by host-pre-scaled W1 stacks.
  * Jacobian trace: trace(J4 J3 J2 J1) with J_l = diag(s_l) W_l,
    s_l = sigmoid(h_l)*g_l (l<4), J4 = diag(g4) W4.  Using per-tangent-dir
    folded weights (eval-independent, host precomputed):
      W2k[k] = W2 diag(W1[:,k]),  W3k[k] = diag(W4[k,:]) W3
    gives   V2raw_k = W2k[k] @ s1            (PE, bf16)
            V2_k   = g4[k] * s2 * V2raw_k    (one PSUM evacuation, DVE)
            T      = sum_k W3k[k] @ V2_k     (PE, bf16, PSUM-accumulated)
            ktr    = sum_m3 s3 * T           (tiny evac + ones-matmul)
    The ones-matmul lhsT carries h*b_s so lp accumulates directly in a
    persistent PSUM bank across all 120 evals; the final -0.5*sum(y^2)
    ones-matmul lands in the same bank.
"""
import math
import numpy as np
import ml_dtypes
from contextlib import ExitStack

import concourse.bass as bass
import concourse.tile as tile
from concourse import bacc, mybir
from concourse import bass_utils

F32 = mybir.dt.float32
F32R = mybir.dt.float32r
BF16 = mybir.dt.bfloat16
AF = mybir.ActivationFunctionType
ALU = mybir.AluOpType

DATA = 32
WIDTH = 256
BATCH = 4096
NCORES = 8
S = BATCH // NCORES          # 512 samples per core
NSTEPS = 10
HSTEP = -0.1                 # integration step (t1 -> t0)

_A = [
    [],
    [0.161],
    [-0.008480655492356989, 0.335480655492357],
    [2.8971530571054935, -6.359448489975075, 4.3622954328695815],
    [5.325864828439257, -11.748883564062828, 7.4955393428898365, -0.09249506636175525],
    [5.86145544294642, -12.92096931784711, 8.159367898576159, -0.071584973281401, -0.028269050394068383],
]
_B = [0.09646076681806523, 0.01, 0.4798896504144996, 1.379008574103742,
      -3.290069515436081, 2.324710524099774]
_C = [0.0, 0.161, 0.327, 0.9, 0.9800255409045097, 1.0]


def _sigmoid(x):
    return 1.0 / (1.0 + np.exp(-x))


def _precompute(params, steps):
    """Host-side constants.  Returns (shared in_map dict, per-block g4 floats).
    Blocks are in PROCESSING order (reversed(params))."""
    nev = steps * 6
    tvals = np.array([1.0 + st * HSTEP + _C[s] * HSTEP
                      for st in range(steps) for s in range(6)], np.float64)

    consts = {}
    g4f = []        # [block][ev][k] python floats for evac immediates
    for bi, block in enumerate(reversed(params)):
        W = [np.asarray(p["W1"], np.float64) for p in block]
        b1 = [np.asarray(p["b1"], np.float64) for p in block]
        W2g = [np.asarray(p["W2"], np.float64)[:, 0] for p in block]
        b2g = [np.asarray(p["b2"], np.float64) for p in block]
        W3c = [np.asarray(p["W3"], np.float64)[:, 0] for p in block]
        g = []
        d = []
        for l in range(4):
            gl = _sigmoid(np.outer(tvals, W2g[l]) + b2g[l][None, :])  # [nev, dout]
            d.append((b1[l][None, :] * gl + np.outer(tvals, W3c[l])))
            g.append(gl)
        W1, W2, W3, W4 = W

        # stage-stacked pre-scaled L1 lhsT [672, 256] (f32r)
        rows = []
        for s in range(6):
            rows.append(W1.T)                                  # coeff 1 (y)
            for j in range(s):
                rows.append((HSTEP * _A[s][j]) * W1.T)
        consts[f"w1stk{bi}"] = np.concatenate(rows, 0).astype(np.float32)

        consts[f"w2T{bi}"] = W2.T.astype(np.float32)           # [256,256]
        consts[f"w3T{bi}"] = W3.T.astype(np.float32)
        consts[f"w4T{bi}"] = W4.T.astype(np.float32)           # [256,32]

        # tangent folded weights, packed for SBUF [128, 32*2*2*128] bf16
        w2k = np.zeros((128, DATA * 2 * 2 * 128), np.float32)
        w3k = np.zeros((128, DATA * 2 * 2 * 128), np.float32)
        for k in range(DATA):
            L2 = W2.T * W1[:, k:k + 1]        # lhsT_k [m1, m2]
            L3 = W3.T * W4[k:k + 1, :]        # lhsT_k [m2, m3]
            for c in range(2):
                for mt in range(2):
                    col = ((k * 2 + c) * 2 + mt) * 128
                    w2k[:, col:col + 128] = L2[c * 128:(c + 1) * 128,
                                               mt * 128:(mt + 1) * 128]
                    w3k[:, col:col + 128] = L3[c * 128:(c + 1) * 128,
                                               mt * 128:(mt + 1) * 128]
        consts[f"w2k{bi}"] = w2k.astype(ml_dtypes.bfloat16)
        consts[f"w3k{bi}"] = w3k.astype(ml_dtypes.bfloat16)

        # gates layers 1-3: [128, 3*2*2*nev] fp32, col ((l*2+mt)*2+c)*nev+ev
        gd = np.zeros((128, 3 * 2 * 2 * nev), np.float32)
        for l in range(3):
            for mt in range(2):
                gd[:, ((l * 2 + mt) * 2 + 0) * nev:((l * 2 + mt) * 2 + 1) * nev] = \
                    g[l][:, mt * 128:(mt + 1) * 128].T
                gd[:, ((l * 2 + mt) * 2 + 1) * nev:((l * 2 + mt) * 2 + 2) * nev] = \
                    d[l][:, mt * 128:(mt + 1) * 128].T
        consts[f"gd{bi}"] = gd

        # layer 4 gates replicated on 4x32 partitions: [128, 2*nev]
        g4d4 = np.zeros((128, 2 * nev), np.float32)
        g4d4[:, 0:nev] = np.tile(g[3].T, (4, 1))
        g4d4[:, nev:2 * nev] = np.tile(d[3].T, (4, 1))
        consts[f"g4d4{bi}"] = g4d4

        g4f.append([[float(g[3][ev, k]) for k in range(DATA)] for ev in range(nev)])

    # shared small constants
    hb = np.zeros((128, 6), np.float32)
    for s in range(6):
        hb[:, s] = HSTEP * _B[s]
    consts["hb"] = hb
    ib = np.zeros((224, DATA), np.float32)      # y-combo identity blocks
    ib[0:DATA, :] = np.eye(DATA)
    for j in range(6):
        ib[DATA * (j + 1):DATA * (j + 2), :] = (HSTEP * _B[j]) * np.eye(DATA)
    consts["ib"] = ib
    consts["nh"] = np.full((DATA, 1), -0.5, np.float32)
    consts["lnc"] = np.full((1, 1), -0.5 * DATA * math.log(2 * math.pi), np.float32)
    return consts, g4f


def _build(g4f, steps, blocks):
    """Build the Bass program.  g4f: [block][ev][k] floats."""
    nev = steps * 6
    nc = bacc.Bacc("TRN2", target_bir_lowering=False, debug=False,
                   num_devices=NCORES)

    yT = nc.dram_tensor("yT", [DATA, S], F32R, kind="ExternalInput").ap()
    dr = {}
    for bi in range(blocks):
        dr[f"w1stk{bi}"] = nc.dram_tensor(f"w1stk{bi}", [672, 256], F32R, kind="ExternalInput").ap()
        dr[f"w2T{bi}"] = nc.dram_tensor(f"w2T{bi}", [256, 256], F32R, kind="ExternalInput").ap()
        dr[f"w3T{bi}"] = nc.dram_tensor(f"w3T{bi}", [256, 256], F32R, kind="ExternalInput").ap()
        dr[f"w4T{bi}"] = nc.dram_tensor(f"w4T{bi}", [256, DATA], F32R, kind="ExternalInput").ap()
        dr[f"w2k{bi}"] = nc.dram_tensor(f"w2k{bi}", [128, 16384], BF16, kind="ExternalInput").ap()
        dr[f"w3k{bi}"] = nc.dram_tensor(f"w3k{bi}", [128, 16384], BF16, kind="ExternalInput").ap()
        dr[f"gd{bi}"] = nc.dram_tensor(f"gd{bi}", [128, 12 * nev], F32, kind="ExternalInput").ap()
        dr[f"g4d4{bi}"] = nc.dram_tensor(f"g4d4{bi}", [128, 2 * nev], F32, kind="ExternalInput").ap()
    hb_d = nc.dram_tensor("hb", [128, 6], F32R, kind="ExternalInput").ap()
    ib_d = nc.dram_tensor("ib", [224, DATA], F32R, kind="ExternalInput").ap()
    nh_d = nc.dram_tensor("nh", [DATA, 1], F32R, kind="ExternalInput").ap()
    lnc_d = nc.dram_tensor("lnc", [1, 1], F32, kind="ExternalInput").ap()
    out_d = nc.dram_tensor("out", [1, S], F32, kind="ExternalOutput").ap()

    stage_rows = [32 * (1 + s) for s in range(6)]
    stage_off = np.cumsum([0] + stage_rows).tolist()

    with tile.TileContext(nc) as tc, ExitStack() as ctx:
        cst = ctx.enter_context(tc.tile_pool(name="cst", bufs=1))
        wbig = ctx.enter_context(tc.tile_pool(name="wbig", bufs=1))
        kyp = ctx.enter_context(tc.tile_pool(name="kyp", bufs=2))
        fw = ctx.enter_context(tc.tile_pool(name="fw", bufs=3))
        ylp = ctx.enter_context(tc.tile_pool(name="ylp", bufs=4))
        sp = ctx.enter_context(tc.tile_pool(name="sp", bufs=4))
        v2p = ctx.enter_context(tc.tile_pool(name="v2p", bufs=4))
        tsp = ctx.enter_context(tc.tile_pool(name="tsp", bufs=4))
        osb = ctx.enter_context(tc.tile_pool(name="osb", bufs=1))

        mmps = ctx.enter_context(tc.tile_pool(name="mmps", bufs=4, space="PSUM"))
        h4ps = ctx.enter_context(tc.tile_pool(name="h4ps", bufs=1, space="PSUM"))
        tps = ctx.enter_context(tc.tile_pool(name="tps", bufs=2, space="PSUM"))
        lpp = ctx.enter_context(tc.tile_pool(name="lpp", bufs=1, space="PSUM"))

        # shared constants
        hb_t = cst.tile([128, 6], F32R, tag="hb")
        nc.sync.dma_start(hb_t[:], hb_d)
        ib1_t = cst.tile([128, DATA], F32R, tag="ib1")
        nc.sync.dma_start(ib1_t[:], ib_d[0:128, :])
        ib2_t = cst.tile([96, DATA], F32R, tag="ib2")
        nc.sync.dma_start(ib2_t[:], ib_d[128:224, :])
        nh_t = cst.tile([DATA, 1], F32R, tag="nh")
        nc.sync.dma_start(nh_t[:], nh_d)
        lnc_t = cst.tile([1, 1], F32, tag="lnc")
        nc.sync.dma_start(lnc_t[:], lnc_d)

        lp_t = lpp.tile([128, S], F32)
        lp_started = [False]

        def lp_mm(lhsT, rhs, stop=False):
            nc.tensor.matmul(lp_t[0:1, :], lhsT, rhs,
                             start=not lp_started[0], stop=stop)
            lp_started[0] = True

        # initial state bank
        ky1 = kyp.tile([128, S], F32R, tag="ky1")
        ky2 = kyp.tile([96, S], F32R, tag="ky2")
        nc.sync.dma_start(ky1[0:DATA, :], yT)

        for bi in range(blocks):
            # ---- load per-block constants ----
            w2k_t = wbig.tile([128, 16384], BF16, tag="w2k")
            nc.sync.dma_start(w2k_t[:], dr[f"w2k{bi}"])
            w3k_t = wbig.tile([128, 16384], BF16, tag="w3k")
            nc.sync.dma_start(w3k_t[:], dr[f"w3k{bi}"])
            gd_t = cst.tile([128, 12 * nev], F32, tag="gd")
            nc.sync.dma_start(gd_t[:], dr[f"gd{bi}"])
            g4d4_t = cst.tile([128, 2 * nev], F32, tag="g4d4")
            nc.sync.dma_start(g4d4_t[:], dr[f"g4d4{bi}"])
            w2T_t = [cst.tile([128, 256], F32R, tag=f"w2T{c}", name=f"w2T_t{c}") for c in range(2)]
            w3T_t = [cst.tile([128, 256], F32R, tag=f"w3T{c}", name=f"w3T_t{c}") for c in range(2)]
            w4T_t = [cst.tile([128, DATA], F32R, tag=f"w4T{c}", name=f"w4T_t{c}") for c in range(2)]
            for c in range(2):
                nc.sync.dma_start(w2T_t[c][:], dr[f"w2T{bi}"][c * 128:(c + 1) * 128, :])
                nc.sync.dma_start(w3T_t[c][:], dr[f"w3T{bi}"][c * 128:(c + 1) * 128, :])
                nc.sync.dma_start(w4T_t[c][:], dr[f"w4T{bi}"][c * 128:(c + 1) * 128, :])
            w1s_t = []
            for s in range(6):
                r = stage_rows[s]
                o = stage_off[s]
                ca = cst.tile([min(r, 128), 256], F32R, tag=f"w1s{s}a")
                nc.sync.dma_start(ca[:], dr[f"w1stk{bi}"][o:o + min(r, 128), :])
                chunks = [ca]
                if r > 128:
                    cb = cst.tile([r - 128, 256], F32R, tag=f"w1s{s}b")
                    nc.sync.dma_start(cb[:], dr[f"w1stk{bi}"][o + 128:o + r, :])
                    chunks.append(cb)
                w1s_t.append(chunks)

            def gcol(l, mt, c, ev):
                return ((l * 2 + mt) * 2 + c) * nev + ev

            for st in range(steps):
                for s in range(6):
                    ev = st * 6 + s
                    # ---------- forward L1 (stage-stacked) ----------
                    rows = stage_rows[s]
                    hcur = [mmps.tile([128, S], F32, tag="mm", name="hcur") for _ in range(2)]
                    for mt in range(2):
                        if rows <= 128:
                            nc.tensor.matmul(hcur[mt][:],
                                             w1s_t[s][0][0:rows, bass.ts(mt, 128)],
                                             ky1[0:rows, :], start=True, stop=True)
                        else:
                            nc.tensor.matmul(hcur[mt][:],
                                             w1s_t[s][0][:, bass.ts(mt, 128)],
                                             ky1[:, :], start=True, stop=False)
                            nc.tensor.matmul(hcur[mt][:],
                                             w1s_t[s][1][0:rows - 128, bass.ts(mt, 128)],
                                             ky2[0:rows - 128, :], start=False, stop=True)

                    svals = []       # s1,s2,s3 bf16 tile pairs
                    for l in range(3):
                        ynext = []
                        sl = []
                        for mt in range(2):
                            g_ap = gd_t[:, gcol(l, mt, 0, ev):gcol(l, mt, 0, ev) + 1]
                            d_ap = gd_t[:, gcol(l, mt, 1, ev):gcol(l, mt, 1, ev) + 1]
                            h_t = fw.tile([128, S], F32, tag="h")
                            nc.vector.tensor_scalar(h_t[:], hcur[mt][:], g_ap, d_ap,
                                                    op0=ALU.mult, op1=ALU.add)
                            nab = fw.tile([128, S], F32, tag="nab")
                            nc.vector.scalar_tensor_tensor(nab[:], h_t[:], -1.0, h_t[:],
                                                           op0=ALU.mult, op1=ALU.min)
                            u_t = fw.tile([128, S], F32, tag="u")
                            nc.scalar.activation(u_t[:], nab[:], AF.Exp)
                            v_t = fw.tile([128, S], F32, tag="v")
                            nc.scalar.activation(v_t[:], u_t[:], AF.Ln, bias=1.0)
                            y_t = ylp.tile([128, S], F32R, tag="yl")
                            nc.vector.scalar_tensor_tensor(y_t[:], h_t[:], 0.0, v_t[:],
                                                           op0=ALU.max, op1=ALU.add)
                            w_t = fw.tile([128, S], F32, tag="w")
                            nc.vector.scalar_tensor_tensor(w_t[:], h_t[:], 0.0, v_t[:],
                                                           op0=ALU.min, op1=ALU.subtract)
                            sg_t = fw.tile([128, S], F32, tag="sg")
                            nc.scalar.activation(sg_t[:], w_t[:], AF.Exp)
                            s_t = sp.tile([128, S], BF16, tag=f"s{l}")
                            nc.vector.tensor_scalar_mul(s_t[:], sg_t[:], g_ap)
                            ynext.append(y_t)
                            sl.append(s_t)
                        svals.append(sl)
                        if l < 2:
                            hnext = [mmps.tile([128, S], F32, tag="mm", name="hnext") for _ in range(2)]
                            wT = w2T_t if l == 0 else w3T_t
                            for mt in range(2):
                                for c in range(2):
                                    nc.tensor.matmul(hnext[mt][:],
                                                     wT[c][:, bass.ts(mt, 128)],
                                                     ynext[c][:],
                                                     start=(c == 0), stop=(c == 1))
                            hcur = hnext
                        else:
                            # ---------- L4 -> f -> KY slot (via SBUF DMA hop) ----------
                            h4t = h4ps.tile([DATA, S], F32, tag="h4")
                            for c in range(2):
                                nc.tensor.matmul(h4t[:, :],
                                                 w4T_t[c][:, 0:DATA],
                                                 ynext[c][:],
                                                 start=(c == 0), stop=(c == 1))
                            f_sb = fw.tile([DATA, S], F32R, tag="fsb")
                            nc.scalar.activation(
                                f_sb[:], h4t[:, :], AF.Identity,
                                scale=g4d4_t[0:DATA, ev:ev + 1],
                                bias=g4d4_t[0:DATA, nev + ev:nev + ev + 1])
                            pos = 32 * (s + 1) if s < 3 else 32 * (s - 3)
                            tgt = ky1[pos:pos + 32, :] if s < 3 else ky2[pos:pos + 32, :]
                            nc.sync.dma_start(tgt, f_sb[:])

                    s1, s2, s3 = svals

                    # ---------- tangent ----------
                    Tt = [tps.tile([128, S], F32, tag="tps", name="Tt") for _ in range(2)]

                    def w2k_group(k):
                        vr = [mmps.tile([128, S], F32, tag="mm", name="vr") for _ in range(2)]
                        for mt in range(2):
                            for c in range(2):
                                col = ((k * 2 + c) * 2 + mt) * 128
                                nc.tensor.matmul(vr[mt][:],
                                                 w2k_t[:, col:col + 128],
                                                 s1[c][:],
                                                 start=(c == 0), stop=(c == 1))
                        return vr

                    vr_cur = w2k_group(0)
                    for k in range(DATA):
                        vr_next = w2k_group(k + 1) if k + 1 < DATA else None
                        v2sb = []
                        for mt in range(2):
                            vt = v2p.tile([128, S], BF16, tag="v2sb")
                            nc.vector.scalar_tensor_tensor(
                                vt[:], vr_cur[mt][:], g4f[bi][ev][k], s2[mt][:],
                                op0=ALU.mult, op1=ALU.mult)
                            v2sb.append(vt)
                        for mt in range(2):
                            for c in range(2):
                                col = ((k * 2 + c) * 2 + mt) * 128
                                nc.tensor.matmul(Tt[mt][:],
                                                 w3k_t[:, col:col + 128],
                                                 v2sb[c][:],
                                                 start=(k == 0 and c == 0),
                                                 stop=(k == DATA - 1 and c == 1))
                        vr_cur = vr_next

                    for mt in range(2):
                        ts_t = tsp.tile([128, S], F32R, tag="ts")
                        nc.vector.tensor_tensor(ts_t[:], Tt[mt][:], s3[mt][:],
                                                op=ALU.mult)
                        lp_mm(hb_t[:, s:s + 1], ts_t[:])

                # ---------- step end: y update ----------
                ky1n = kyp.tile([128, S], F32R, tag="ky1")
                ky2n = kyp.tile([96, S], F32R, tag="ky2")
                ycp = mmps.tile([128, S], F32, tag="mm")
                nc.tensor.matmul(ycp[0:DATA, :], ib1_t[:, :], ky1[:, :],
                                 start=True, stop=False)
                nc.tensor.matmul(ycp[0:DATA, :], ib2_t[:, :], ky2[:, :],
                                 start=False, stop=True)
                nc.vector.tensor_copy(ky1n[0:DATA, :], ycp[0:DATA, :])
                ky1, ky2 = ky1n, ky2n

        # ---------- finalization ----------
        ysq = osb.tile([DATA, S], F32R, tag="ysq")
        nc.vector.tensor_tensor(ysq[:], ky1[0:DATA, :], ky1[0:DATA, :], op=ALU.mult)
        lp_mm(nh_t[:, 0:1], ysq[:], stop=True)
        out_t = osb.tile([1, S], F32, tag="out")
        nc.scalar.activation(out_t[:], lp_t[0:1, :], AF.Identity,
                             bias=lnc_t[:])
        nc.sync.dma_start(out_d, out_t[:])

    nc.compile()
    return nc


def _run(y, params, steps=NSTEPS, blocks=2, trace=False):
    consts, g4f = _precompute(params, steps)
    nc = _build(g4f, steps, blocks)
    yT = np.ascontiguousarray(np.asarray(y, np.float32).T)    # [32, 4096]
    in_maps = []
    for c in range(NCORES):
        m = dict(consts)
        # drop unused block tensors if blocks < 2
        m = {k: v for k, v in m.items()
             if not (k[-1].isdigit() and int(k[-1]) >= blocks)}
        m["yT"] = np.ascontiguousarray(yT[:, c * S:(c + 1) * S])
        in_maps.append(m)
    res = bass_utils.run_bass_kernel_spmd(
        nc, in_maps, core_ids=list(range(NCORES)), trace=trace)
    out = np.concatenate([res.results[c]["out"][0] for c in range(NCORES)])
    return out.astype(np.float32), res


def kernel(y, params):
    out, _ = _run(np.asarray(y), params)
    return out
